# revision 10
# baseline (speedup 1.0000x reference)
"""Trainium2 Bass kernel for MLA-style causal attention (nn_CausalAttention).

Sharding: tensor-parallel over heads x data-parallel over batch.
8 cores = 2 batch groups x 4 cores; each core handles 1 batch element and 4
heads.  Host prep transposes x (the PE contracts over the partition dim, so
all activations are kept in [feature, token] layout on chip), slices the
up-projection weights by head columns and Wo by head rows, and precomputes
transposed rope tables plus a causal staircase mask.  Each core computes a
partial (D, T) output through its own Wo row-block; the host sums the four
partials per batch and transposes back.  No device collectives needed.

All matmuls run in float32r (fp22-truncated fp32 read) which streams at the
full PE rate for free dims >= 256, with fp32 PSUM accumulation.
"""

import sys

if '/opt/trn_rl_repo' not in sys.path:
    sys.path.insert(0, '/opt/trn_rl_repo')

from contextlib import ExitStack

import numpy as np

import concourse.bass as bass  # noqa: F401  (engine types resolve through bacc)
import concourse.tile as tile
from concourse import bacc, mybir
from concourse.bass_utils import run_bass_kernel_spmd

F32 = mybir.dt.float32
F32R = mybir.dt.float32r
AF = mybir.ActivationFunctionType

B, T, D = 2, 2048, 2048
H, LAT, RD = 16, 512, 64
HD = D // H                    # 128
HPC = H // 4                   # 4 heads per core
N_CORES = 8
QB = 512                       # query block (tq)
NQB = T // QB                  # 4
CH = 128                       # key chunk (tk)
NCH = T // CH                  # 16
WC_COLS = LAT + LAT + RD       # 1088 = [Wqd | Wkvd | Wkr]
SCALE = 1.0 / float(np.sqrt(HD + RD))

_prog_cache = {}
last_results = None            # test.py reads exec_time_ns from here


def _build_program():
    nc = bacc.Bacc("TRN2", target_bir_lowering=False, debug=False,
                   num_devices=N_CORES)

    xT = nc.dram_tensor("xT", [D, T], F32, kind="ExternalInput").ap()
    wc = nc.dram_tensor("wc", [D, WC_COLS], F32, kind="ExternalInput").ap()
    wqu = nc.dram_tensor("wqu", [LAT, HPC * HD], F32, kind="ExternalInput").ap()
    wqr = nc.dram_tensor("wqr", [LAT, HPC * RD], F32, kind="ExternalInput").ap()
    wku = nc.dram_tensor("wku", [LAT, HPC * HD], F32, kind="ExternalInput").ap()
    wvu = nc.dram_tensor("wvu", [LAT, HPC * HD], F32, kind="ExternalInput").ap()
    wo = nc.dram_tensor("wo", [HPC * HD, D], F32, kind="ExternalInput").ap()
    cos2 = nc.dram_tensor("cos2", [128, T], F32, kind="ExternalInput").ap()
    onesd = nc.dram_tensor("onesd", [128, 128], F32, kind="ExternalInput").ap()
    sin2 = nc.dram_tensor("sin2", [128, T], F32, kind="ExternalInput").ap()
    yT = nc.dram_tensor("yT", [D, T], F32, kind="ExternalOutput").ap()

    # DRAM views with 128-row chunks moved to the partition dim
    wc_r = wc.rearrange("(n p) m -> p n m", p=128)     # [128, 16, 1088]
    wqu_r = wqu.rearrange("(n p) m -> p n m", p=128)   # [128, 4, 512]
    wqr_r = wqr.rearrange("(n p) m -> p n m", p=128)   # [128, 4, 256]
    wku_r = wku.rearrange("(n p) m -> p n m", p=128)
    wvu_r = wvu.rearrange("(n p) m -> p n m", p=128)
    wo_r = wo.rearrange("(n p) m -> p n m", p=128)     # [128, 4, 2048]

    with tile.TileContext(nc) as tc, ExitStack() as ctx:
        const = ctx.enter_context(tc.tile_pool(name="const", bufs=1))
        persist = ctx.enter_context(tc.tile_pool(name="persist", bufs=1))
        xtp = ctx.enter_context(tc.tile_pool(name="xtp", bufs=1))
        wcp = ctx.enter_context(tc.tile_pool(name="wcp", bufs=2))
        blk = ctx.enter_context(tc.tile_pool(name="blk", bufs=1))
        jit = ctx.enter_context(tc.tile_pool(name="jit", bufs=2))
        probsp = ctx.enter_context(tc.tile_pool(name="probsp", bufs=2))
        attnp = ctx.enter_context(tc.tile_pool(name="attnp", bufs=1))
        wop = ctx.enter_context(tc.tile_pool(name="wop", bufs=2))
        outp = ctx.enter_context(tc.tile_pool(name="outp", bufs=2))
        psum = ctx.enter_context(tc.tile_pool(name="psum", bufs=8, space="PSUM"))

        def ps_tile():
            return psum.tile([128, QB], F32, tag="ps", name="ps")

        # constants
        ones_sb = const.tile([128, 128], F32R)
        nc.sync.dma_start(ones_sb[:], onesd[:].bitcast(F32R))

        # persistent per-core tensors
        kT_all = persist.tile([128, HPC, T], F32R)       # K^T  [hd, h, tk]
        v_all = persist.tile([128, NCH, HPC * HD], F32R)  # V    [tk_in, tkc, hcols]
        krT = persist.tile([128, T], F32R)               # roped kr^T, rows 0:64 and 64:128 identical
        wqu_sb = persist.tile([128, 4, HPC * HD], F32R)
        wqr_sb = persist.tile([128, 4, HPC * RD], F32R)
        wku_sb = persist.tile([128, 4, HPC * HD], F32R)
        wvu_sb = persist.tile([128, 4, HPC * HD], F32R)
        nc.sync.dma_start(wqu_sb[:], wqu_r.bitcast(F32R))
        nc.sync.dma_start(wqr_sb[:], wqr_r.bitcast(F32R))
        nc.sync.dma_start(wku_sb[:], wku_r.bitcast(F32R))
        nc.sync.dma_start(wvu_sb[:], wvu_r.bitcast(F32R))

        # down-projection output column chunks: 4x ql, 4x ckv, 1x kr
        M_SZ = [128] * 8 + [64]
        M_OFF = [128 * i for i in range(8)] + [1024]

        def rope_inplace(dst, src, cos_t, sin_t, p0, p1, tag):
            """dst[p0:p1] = src[p0:p1]*cos + rotate_half(src[p0:p1])*sin.

            p1-p0 must be 64 (one rope head block).  sin_t rows carry the
            rotate_half sign fold.  src/dst may alias only if dst != src rows.
            """
            shf = jit.tile([128, QB], F32R, tag="shf", bufs=1)
            h0, h1, mid = p0, p1, (p0 + p1) // 2
            nc.sync.dma_start(shf[h0:mid, :], src[mid:h1, :])
            nc.sync.dma_start(shf[mid:h1, :], src[h0:mid, :])
            t1 = jit.tile([128, QB], F32R, tag="t1", bufs=1)
            nc.vector.tensor_mul(t1[h0:h1, :], src[h0:h1, :], cos_t[h0:h1, :])
            nc.vector.tensor_mul(shf[h0:h1, :], shf[h0:h1, :], sin_t[h0:h1, :])
            nc.vector.tensor_add(dst[h0:h1, :], t1[h0:h1, :], shf[h0:h1, :])

        for qb in range(NQB):
            qs = slice(qb * QB, (qb + 1) * QB)

            # rope table slices for this block
            cos_t = jit.tile([128, QB], F32R, tag="cos", bufs=1)
            sin_t = jit.tile([128, QB], F32R, tag="sin", bufs=1)
            nc.sync.dma_start(cos_t[:], cos2[:, qs].bitcast(F32R))
            nc.sync.dma_start(sin_t[:], sin2[:, qs].bitcast(F32R))

            # ---- down-projection: [ql | ckv | kr]^T for this block ----
            xt = xtp.tile([128, 16, QB], F32R, tag="xt")
            nc.sync.dma_start(xt[:], xT[:, qs].rearrange("(n p) m -> p n m", p=128).bitcast(F32R))

            ql_blk = blk.tile([128, 4, QB], F32R, tag="ql")
            ckv_blk = blk.tile([128, 4, QB], F32R, tag="ckv")
            for m in range(9):
                wc_t = wcp.tile([128, 16, 128], F32R, tag="wc")
                nc.sync.dma_start(
                    wc_t[:, :, 0:M_SZ[m]],
                    wc_r[:, :, M_OFF[m]:M_OFF[m] + M_SZ[m]].bitcast(F32R))
                ps = ps_tile()
                for d in range(16):
                    nc.tensor.matmul(ps[0:M_SZ[m], :], wc_t[:, d, 0:M_SZ[m]],
                                     xt[:, d, :], start=(d == 0), stop=(d == 15))
                if m < 4:
                    nc.vector.tensor_copy(ql_blk[:, m, :], ps[:])
                elif m < 8:
                    nc.vector.tensor_copy(ckv_blk[:, m - 4, :], ps[:])
                else:
                    nc.vector.tensor_copy(krT[0:64, qs], ps[0:64, :])

            # kr rope on rows 0:64
            rope_inplace(krT[:, qs], krT[:, qs], cos_t, sin_t, 0, 64, "kr")

            # ---- k / v up-projections for this block ----
            for h in range(HPC):
                ps = ps_tile()
                for lc in range(4):
                    nc.tensor.matmul(ps[:], wku_sb[:, lc, h * HD:(h + 1) * HD],
                                     ckv_blk[:, lc, :], start=(lc == 0), stop=(lc == 3))
                nc.vector.tensor_copy(kT_all[:, h, qs], ps[:])
            for i in range(4):
                tkc = 4 * qb + i
                ps = ps_tile()
                for lc in range(4):
                    nc.tensor.matmul(ps[:], ckv_blk[:, lc, i * CH:(i + 1) * CH],
                                     wvu_sb[:, lc, :], start=(lc == 0), stop=(lc == 3))
                nc.vector.tensor_copy(v_all[:, tkc, :], ps[:])

            # ---- attention for this query block ----
            attnT = attnp.tile([128, HPC, QB], F32R, tag="attnT")
            nchunks = 4 * (qb + 1)
            for h in range(HPC):
                # qr up-projection (JIT)
                psr = ps_tile()
                for lc in range(4):
                    nc.tensor.matmul(psr[0:64, :],
                                     wqr_sb[:, lc, h * RD:(h + 1) * RD],
                                     ql_blk[:, lc, :], start=(lc == 0), stop=(lc == 3))
                qr_raw = jit.tile([128, QB], F32R, tag="qr_raw", bufs=1)
                nc.vector.tensor_copy(qr_raw[0:64, :], psr[0:64, :])
                qr_rt = jit.tile([128, QB], F32R, tag="qr_rt")
                rope_inplace(qr_rt, qr_raw, cos_t, sin_t, 0, 64, "q")

                # q up-projection (JIT)
                psq = ps_tile()
                for lc in range(4):
                    nc.tensor.matmul(psq[:], wqu_sb[:, lc, h * HD:(h + 1) * HD],
                                     ql_blk[:, lc, :], start=(lc == 0), stop=(lc == 3))
                qT_t = jit.tile([128, QB], F32R, tag="qT")
                nc.vector.tensor_copy(qT_t[:], psq[:])

                po = ps_tile()
                pd = ps_tile()
                for i in range(nchunks):
                    ks = slice(i * CH, (i + 1) * CH)
                    s = ps_tile()
                    nc.tensor.matmul(s[:], kT_all[:, h, ks], qT_t[:],
                                     start=True, stop=False)
                    nc.tensor.matmul(s[:], krT[0:64, ks], qr_rt[0:64, :],
                                     start=False, stop=True)
                    pr = probsp.tile([128, QB], F32R, tag="probs")
                    nc.scalar.activation(pr[:], s[:], AF.Exp, scale=SCALE)
                    if i >= 4 * qb:
                        # zero probs where key pos > query pos:
                        # keep iff j - i_part - r >= 0
                        r = (i - 4 * qb) * CH
                        nc.gpsimd.affine_select(
                            out=pr[:], in_=pr[:],
                            pattern=[[1, QB]],
                            compare_op=mybir.AluOpType.is_ge,
                            fill=0.0, base=-r, channel_multiplier=-1)
                    nc.tensor.matmul(po[:], v_all[:, i, h * HD:(h + 1) * HD],
                                     pr[:], start=(i == 0), stop=(i == nchunks - 1))
                    nc.tensor.matmul(pd[:], ones_sb[:], pr[:],
                                     start=(i == 0), stop=(i == nchunks - 1))
                rec = probsp.tile([128, QB], F32R, tag="probs")
                with nc.allow_low_precision("f32r softmax denominator is ample"):
                    nc.vector.reciprocal(rec[:], pd[:])
                nc.vector.tensor_mul(attnT[:, h, :], po[:], rec[:])

            # ---- Wo partial for this query block ----
            for dc in range(16):
                wo_t = wop.tile([128, 4, 128], F32R, tag="wo")
                nc.sync.dma_start(wo_t[:], wo_r[:, :, dc * 128:(dc + 1) * 128].bitcast(F32R))
                pw = ps_tile()
                for hc in range(4):
                    nc.tensor.matmul(pw[:], wo_t[:, hc, :], attnT[:, hc, :],
                                     start=(hc == 0), stop=(hc == 3))
                out_t = outp.tile([128, QB], F32, tag="out")
                nc.scalar.copy(out_t[:], pw[:])
                nc.sync.dma_start(yT[dc * 128:(dc + 1) * 128, qs], out_t[:])

    nc.compile()
    return nc


def _host_prep(x, Wqd, Wqu, Wqr, Wkvd, Wku, Wvu, Wkr, Wo):
    f32 = np.float32
    # rope tables, transposed and sign-folded, duplicated across both
    # 64-partition halves so a packed head pair ropes in one pass
    freqs = 1.0 / 10000.0 ** (np.arange(0, RD, 2, dtype=f32) / RD)
    t = np.arange(T, dtype=f32)
    ang = np.outer(t, freqs)                       # (T, 32)
    cos = np.tile(np.cos(ang), (1, 2)).T           # (64, T)
    sin = np.tile(np.sin(ang), (1, 2)).T
    sinS = sin.copy()
    sinS[:RD // 2] = -sinS[:RD // 2]
    cos2 = np.ascontiguousarray(np.concatenate([cos, cos], axis=0), dtype=f32)
    sin2 = np.ascontiguousarray(np.concatenate([sinS, sinS], axis=0), dtype=f32)

    wc_full = np.ascontiguousarray(
        np.concatenate([Wqd, Wkvd, Wkr], axis=1), dtype=f32)   # (D, 1088)

    in_maps = []
    for c in range(N_CORES):
        b, r = divmod(c, 4)
        in_maps.append({
            "xT": np.ascontiguousarray(x[b].T, dtype=f32),
            "wc": wc_full,
            "wqu": np.ascontiguousarray(Wqu[:, r * HPC * HD:(r + 1) * HPC * HD], dtype=f32),
            "wqr": np.ascontiguousarray(Wqr[:, r * HPC * RD:(r + 1) * HPC * RD], dtype=f32),
            "wku": np.ascontiguousarray(Wku[:, r * HPC * HD:(r + 1) * HPC * HD], dtype=f32),
            "wvu": np.ascontiguousarray(Wvu[:, r * HPC * HD:(r + 1) * HPC * HD], dtype=f32),
            "wo": np.ascontiguousarray(Wo[r * HPC * HD:(r + 1) * HPC * HD, :], dtype=f32),
            "cos2": cos2,
            "onesd": np.ones((128, 128), dtype=f32),
            "sin2": sin2,
        })
    return in_maps


def kernel(x, Wqd, Wqu, Wqr, Wkvd, Wku, Wvu, Wkr, Wo):
    global last_results
    x = np.asarray(x, dtype=np.float32)
    args = [np.asarray(a, dtype=np.float32)
            for a in (Wqd, Wqu, Wqr, Wkvd, Wku, Wvu, Wkr, Wo)]

    if "nc" not in _prog_cache:
        _prog_cache["nc"] = _build_program()
    nc = _prog_cache["nc"]

    in_maps = _host_prep(x, *args)
    res = run_bass_kernel_spmd(nc, in_maps, list(range(N_CORES)))
    last_results = res

    out = np.empty((B, T, D), dtype=np.float32)
    for b in range(B):
        acc = np.zeros((D, T), dtype=np.float64)
        for r in range(4):
            acc += res.results[4 * b + r]["yT"]
        out[b] = acc.T.astype(np.float32)
    return out
